# revision 70
# baseline (speedup 1.0000x reference)
"""Causal multi-head attention on 8 Trainium2 NeuronCores.

Sharding: Megatron-style tensor parallelism over heads. Each of the 8
cores computes 2 of the 16 heads end-to-end:
  - column-parallel Q/K/V projections (each core owns a 256-feature slice
    of wq/wk/wv),
  - per-head causal attention entirely on-core,
  - row-parallel output projection producing a partial [B*S, D] output.
The 8 partials are summed on the host (the "all-reduce") and bo added.

Numerics / engine layout:
  - Q/K/V projections run on the PE in fp8e4 DoubleRow perf mode (256-deep
    contraction at 0.5 cycles/col) with 3-term error compensation:
    x8*w8 + x8*wr + xr*w8 where x8/xr and w8/wr are fp8 value/residual
    pairs.  Residual error ~0.15%, at 75% of the bf16 cost.
  - K's bias is dropped: it adds a per-query constant to each score row,
    which softmax cancels exactly.  Q keeps its bias (folded into the
    PSUM drain); V's bias is folded into the host-side output bias.
  - Scores are computed transposed ([k, q]) in bf16 — score noise is
    amplified ~1:1 into the max-norm metric by rows with competing top
    keys, so the score path needs >=8 mantissa bits.  For query chunks
    c>=1 the exp'd probabilities are written as fp8e4 with a per-chunk
    shift (numerator and denominator share it, so it cancels), and
    PV / rowsum run as fp8 DoubleRow matmuls contracting 256 keys per
    instruction.  Chunk 0 keeps a bf16 e (short rows underflow fp8).
  - The output projection is fp8 DoubleRow pairing the two heads with
    full 3-term compensation: y = (a8+ar)@wo8 + a8@wor where (a8, ar)
    is the fp8 value/residual pair of the normalized attention and
    (wo8, wor) the host-side fp8 pair of 32*wo.  1/32 rides in the y
    drain.  PSUM drains run on DVE/ACT only (GPSIMD cannot access PSUM
    on TRN2); Pool handles SBUF-side work: the post-exp causal-mask
    affine_selects, the fp8 (a8, ar) attention splits, and memsets.
  - The schedule is a greedy list-scheduler over fine-grained units
    (QKV accumulation groups, 3-block score groups, per-head PV, outproj
    token tiles) with virtual PE/ACT/DVE clocks and producer-latency
    guards, so the in-order PE stream never outruns ACT exp or the DVE
    drains; x/w DRAM layouts are packed for >=512B DMA descriptors and
    y writes are merged to full rows (serial-DMA issue cost).
"""

import math

import numpy as np

B = 2
S = 2048
D = 2048
H = 16
HD = 128  # head dim
N_CORES = 8
H_LOC = H // N_CORES       # 2 heads per core
F_LOC = H_LOC * HD         # 256 local features per core
KT = D // 128              # 16 contraction tiles
KP = KT // 2               # 8 DoubleRow contraction pairs
CHUNK = 512                # token chunk (matmul moving dim)
NCH = S // CHUNK           # 4 chunks per batch
TT = S // 128              # 16 token tiles per batch

# per-chunk exp shift for the fp8 probability path (c=0 unused: bf16 path)
C_SHIFT = [0.0, 4.5, 4.5, 5.0]
# host-side weight scaling keeps fp8 weight residuals out of subnormals.
W_SCALE = 32.0

PV_TERMS = 2      # e*(v8 [+ vr])

_CACHE = {}


def _build(reps=None):
    import concourse.mybir as mybir
    import concourse.tile as tile
    from concourse import bacc

    F32 = mybir.dt.float32
    BF16 = mybir.dt.bfloat16
    FP8E4 = mybir.dt.float8e4
    DRM = mybir.MatmulPerfMode.DoubleRow
    ADD = mybir.AluOpType.add
    SUB = mybir.AluOpType.subtract
    MULT = mybir.AluOpType.mult
    EXP = mybir.ActivationFunctionType.Exp
    INV_SQRT_HD = 1.0 / math.sqrt(HD)

    nc = bacc.Bacc("TRN2", target_bir_lowering=False, debug=False,
                   num_devices=N_CORES)

    xiT_d = nc.dram_tensor("xiT", [D, 2 * B * S], FP8E4,
                           kind="ExternalInput")
    w_d = {}
    for nm in ("wq", "wk", "wv"):
        w_d[nm] = nc.dram_tensor(nm + "pT", [D, 2 * F_LOC], FP8E4,
                                 kind="ExternalInput")
    woT_d = nc.dram_tensor("woT", [F_LOC, 2 * D], FP8E4,
                           kind="ExternalInput")
    bq_d = nc.dram_tensor("bq2", [HD, H_LOC], F32, kind="ExternalInput")
    ones_d = nc.dram_tensor("ones", [128, 2 * 128], FP8E4,
                            kind="ExternalInput")
    y_d = nc.dram_tensor("y", [B * S, D], BF16, kind="ExternalOutput")

    with tile.TileContext(nc) as tc:
        cpool = tc.alloc_tile_pool(name="const", bufs=1)
        wpool = tc.alloc_tile_pool(name="w", bufs=1)
        xkpool = tc.alloc_tile_pool(name="xk", bufs=3)
        actpool = tc.alloc_tile_pool(name="act", bufs=6)
        ypool = tc.alloc_tile_pool(name="y", bufs=3)
        ripool = tc.alloc_tile_pool(name="ri", bufs=3)
        psq = tc.alloc_tile_pool(name="ps", bufs=3, space="PSUM")
        pss = tc.alloc_tile_pool(name="pss", bufs=3, space="PSUM")
        psa = psr = psq
        pso = tc.alloc_tile_pool(name="pso", bufs=2, space="PSUM")
        qtag = "ps"

        def kview(t, width=CHUNK):
            """[128, KT*width] tile -> [128, KT, width] k-tile view."""
            return t[:].rearrange("p (k f) -> p k f", k=KT)

        # --- loads; DMA queue order is deliberate (wq + first x chunks
        # first so the projection matmuls start a few us in) ---
        def load_w(nm, dram, eng=None):
            """[128, KT, 2, F_LOC] packed (w8 | wr) pair tile."""
            w_t = wpool.tile([128, KT * 2 * F_LOC], FP8E4, tag=nm, name=nm)
            (eng or nc.sync).dma_start(
                w_t[:].rearrange("p (k f) -> p k f", k=KT),
                dram.ap().rearrange("(k p) f -> p k f", p=128),
            )
            return w_t

        def load_x(b, c, split=1):
            """Load one 512-token chunk of packed x.  DRAM rows are
            (k-tile, partition) and each row holds (x8 | xr) for the
            chunk contiguously -> 1KiB descriptors (full DMA rate)."""
            x_t = xkpool.tile([128, KT * 2 * CHUNK], FP8E4, tag="xk",
                              name="x_t")
            col0 = (b * NCH + c) * 2 * CHUNK
            kstep = KT // split
            for s in range(split):
                k0 = s * kstep
                nc.sync.dma_start(
                    x_t[:, k0 * 2 * CHUNK:(k0 + kstep) * 2 * CHUNK]
                        .rearrange("p (k f) -> p k f", k=kstep),
                    xiT_d.ap()[k0 * 128:(k0 + kstep) * 128,
                               col0:col0 + 2 * CHUNK]
                        .rearrange("(k p) f -> p k f", p=128),
                )
            return x_t

        # startup order: wq -> x(c0) -> wk/wv so the first projection
        # matmuls start as early as possible
        w_ts = {"wq": load_w("wq", w_d["wq"])}
        bq_t = cpool.tile([HD, H_LOC], F32, tag="bq")
        nc.sync.dma_start(bq_t[:], bq_d.ap())
        x_first = load_x(0, 0, split=4)
        w_ts["wk"] = load_w("wk", w_d["wk"])
        w_ts["wv"] = load_w("wv", w_d["wv"])
        x_second = load_x(0, 1, split=2)
        ones8 = cpool.tile([128, 2 * 128], FP8E4, tag="ones8")
        nc.sync.dma_start(ones8[:], ones_d.ap())
        # bf16 "ones" for the chunk-0 rowsum (scaled to match the scaled v)
        ones16 = cpool.tile([128, 128], BF16, tag="ones16")
        nc.gpsimd.memset(ones16[:], W_SCALE)
        # per-chunk exp-shift bias tiles (activation bias must be an AP)
        shift_t = {}
        for c in range(1, NCH):
            sh = cpool.tile([128, 1], F32, tag=f"shift{c}", name="sh")
            nc.gpsimd.memset(sh[:], -C_SHIFT[c])
            shift_t[c] = sh
        # warm the ACT Exp table during the QKV phase so the table load
        # doesn't land on the first attention chunk
        warm_t = cpool.tile([128, 1], F32, tag="warm")
        nc.scalar.activation(warm_t[:], bq_t[:, 0:1], EXP,
                             bias=0.0, scale=1.0)
        # PE warm-up: dummy matmuls on an early memset tile bridge the DMA
        # lead-in so the tensor engine's clock is fully ramped (the p-state
        # model needs ~3us of continuous execution) when x/w arrive.
        warm_src = cpool.tile([128, 128], BF16, tag="warmsrc")
        nc.vector.memset(warm_src[:], 0.0)
        warm_ps = psq.tile([128, 128], F32, tag=qtag, name="warm_ps")
        for _ in range(128):
            nc.tensor.matmul(warm_ps[:], warm_src[:], warm_src[:],
                             start=True, stop=True)

        def qkv_group(x_t, c, acts, kind, idx, fused=False):
            """One Q/K/V projection accumulation group (fp8 DR, 3-term).

            kind 'q'/'k' with idx=head; kind 'v' with idx=t4 token tile.
            fused=True runs BOTH heads k-pair-major so the first chunk's
            matmuls consume the streaming x parts at DMA pace."""
            xk = x_t[:].rearrange("p (k two f) -> p k two f", k=KT, two=2)
            qT_t, kT_t, v8_t, vr_t = acts[:4]
            if fused:
                nm = "wq" if kind == "q" else "wk"
                wpv = w_ts[nm][:].rearrange(
                    "p (k two f) -> p k two f", k=KT, two=2)
                w8v = wpv[:, :, 0, :]
                wrv = wpv[:, :, 1, :]
                terms = [(w8v, 0), (wrv, 0), (w8v, 1)]
                pss_h = [psq.tile([128, CHUNK], F32, tag=qtag, name="q_ps")
                         for _ in range(H_LOC)]
                for k2 in range(KP):
                    for h in range(H_LOC):
                        hs = slice(h * HD, (h + 1) * HD)
                        for ti, (wv_, sel) in enumerate(terms):
                            nc.tensor.matmul(
                                pss_h[h][:],
                                wv_[:, 2 * k2:2 * k2 + 2, hs],
                                xk[:, 2 * k2:2 * k2 + 2, sel, :],
                                start=(k2 == 0 and ti == 0),
                                stop=(k2 == KP - 1 and ti == 2),
                                perf_mode=DRM,
                            )
                for h in range(H_LOC):
                    dslice = (qT_t if kind == "q" else kT_t)[
                        :, h * S + c * CHUNK: h * S + (c + 1) * CHUNK]
                    if kind == "q":
                        nc.vector.tensor_scalar(
                            dslice, pss_h[h][:], 1.0 / W_SCALE,
                            bq_t[:, h:h + 1], op0=MULT, op1=ADD)
                    else:
                        nc.vector.tensor_scalar(
                            dslice, pss_h[h][:], 1.0 / W_SCALE, None,
                            op0=MULT)
                return
            if kind in ("q", "k"):
                h = idx
                nm = "wq" if kind == "q" else "wk"
                wpv = w_ts[nm][:].rearrange(
                    "p (k two f) -> p k two f", k=KT, two=2)
                w8v = wpv[:, :, 0, :]
                wrv = wpv[:, :, 1, :]
                hs = slice(h * HD, (h + 1) * HD)
                q_ps = psq.tile([128, CHUNK], F32, tag=qtag, name="q_ps")
                terms = [(w8v, 0), (wrv, 0), (w8v, 1)]
                n = 3 * KP
                i = 0
                for wv_, sel in terms:
                    for k2 in range(KP):
                        nc.tensor.matmul(
                            q_ps[:],
                            wv_[:, 2 * k2:2 * k2 + 2, hs],
                            xk[:, 2 * k2:2 * k2 + 2, sel, :],
                            start=(i == 0), stop=(i == n - 1),
                            perf_mode=DRM,
                        )
                        i += 1
                dslice = (qT_t if kind == "q" else kT_t)[
                    :, h * S + c * CHUNK: h * S + (c + 1) * CHUNK]
                if kind == "q":
                    nc.vector.tensor_scalar(
                        dslice, q_ps[:], 1.0 / W_SCALE,
                        bq_t[:, h:h + 1], op0=MULT, op1=ADD)
                else:
                    # k bias is softmax-invariant: plain scaled copy
                    nc.vector.tensor_scalar(
                        dslice, q_ps[:], 1.0 / W_SCALE, None, op0=MULT)
            else:
                t4 = idx
                wpv = w_ts["wv"][:].rearrange(
                    "p (k two f) -> p k two f", k=KT, two=2)
                w8v = wpv[:, :, 0, :]
                wrv = wpv[:, :, 1, :]
                tt = c * (CHUNK // 128) + t4
                ts4 = slice(t4 * 128, (t4 + 1) * 128)
                v_ps = psq.tile([128, CHUNK], F32, tag=qtag, name="v_ps")
                terms = [(0, w8v), (0, wrv), (1, w8v)]
                n = 3 * KP
                i = 0
                for sel, wv_ in terms:
                    for k2 in range(KP):
                        nc.tensor.matmul(
                            v_ps[:, 0:F_LOC],
                            xk[:, 2 * k2:2 * k2 + 2, sel, ts4],
                            wv_[:, 2 * k2:2 * k2 + 2, :],
                            start=(i == 0), stop=(i == n - 1),
                            perf_mode=DRM,
                        )
                        i += 1
                nc.vector.tensor_copy(
                    v8_t[:, tt * F_LOC:(tt + 1) * F_LOC], v_ps[:, 0:F_LOC])
                nc.vector.tensor_tensor(
                    vr_t[:, tt * F_LOC:(tt + 1) * F_LOC],
                    v_ps[:, 0:F_LOC],
                    v8_t[:, tt * F_LOC:(tt + 1) * F_LOC], SUB)

        def new_e(c):
            if c == 0:
                return [xkpool.tile([128, 4 * CHUNK], BF16, tag="e0",
                                    bufs=2, name="e_t")
                        for _ in range(H_LOC)]
            return [xkpool.tile([128, KT * CHUNK], FP8E4, tag="e8",
                                bufs=4, name="e_t")
                    for _ in range(H_LOC)]

        def sc_group(c, acts, h, blk0, e_t, nblk):
            """Score blocks blk0..blk0+nblk for one head + exp (bf16 PE)."""
            qT_t, kT_t = acts[:2]
            nki = 4 * c + 4
            shift = shift_t.get(c)
            q0 = h * S + c * CHUNK
            for ki in range(blk0, blk0 + nblk):
                r = ki - 4 * c
                trim = 128 * r if r > 0 else 0
                ncol = CHUNK - trim
                diag = ki >= 4 * c
                s_ps = pss.tile([128, CHUNK], F32, tag="pss", name="s_ps")
                nc.tensor.matmul(
                    s_ps[:, 0:ncol],
                    kT_t[:, h * S + ki * 128: h * S + (ki + 1) * 128],
                    qT_t[:, q0 + trim: q0 + CHUNK],
                    start=True, stop=True,
                )
                esl = e_t[:, ki * CHUNK + trim:(ki + 1) * CHUNK]
                nc.scalar.activation(
                    esl, s_ps[:, 0:ncol], EXP,
                    bias=(0.0 if c == 0 else shift[:]),
                    scale=INV_SQRT_HD)
                if diag:
                    # causal mask: zero e where query < key (Pool, off
                    # the PE critical path); keep col j iff trim+j >= p
                    nc.gpsimd.affine_select(
                        out=esl, in_=esl,
                        compare_op=mybir.AluOpType.is_ge,
                        fill=0.0, base=0, pattern=[[1, ncol]],
                        channel_multiplier=-1,
                    )
            if c >= 1 and blk0 == 0:
                # zero the sub-diagonal strips of the odd diagonal blocks
                # (lanes inside the DoubleRow pair's span that exp never
                # writes); done up front, off the PV critical path
                nc.gpsimd.memset(
                    e_t[:, (4 * c + 1) * CHUNK:(4 * c + 1) * CHUNK + 128],
                    0.0)
                nc.gpsimd.memset(
                    e_t[:, (4 * c + 3) * CHUNK + 256:
                        (4 * c + 3) * CHUNK + 384],
                    0.0)

        def pv_head(c, acts, e_t, h, tail_pv=False):
            """Rowsum + PV + normalize for one chunk and head.

            The rowsum runs first so the DVE reciprocal overlaps the PV
            matmuls and the (a8, ar) drain starts as soon as at_ps stops."""
            v8_t, vr_t = acts[2], acts[3]
            at_ps = psr.tile([128, CHUNK], F32, tag=qtag, name="at_ps")
            rs_ps = psr.tile([128, CHUNK], F32, tag=qtag, name="rs_ps")
            ri_t = ripool.tile([128, CHUNK], F32, tag="ri", name="ri_t")
            if c == 0:
                nki = 4
                for ki in range(nki):
                    trim = 128 * ki
                    nc.tensor.matmul(
                        rs_ps[:, trim:CHUNK],
                        ones16[:],
                        e_t[:, ki * CHUNK + trim:(ki + 1) * CHUNK],
                        start=(ki == 0), stop=(ki == nki - 1),
                    )
                nc.vector.reciprocal(ri_t[:], rs_ps[:])
                for term, v_t in enumerate((v8_t, vr_t)):
                    for ki in range(nki):
                        trim = 128 * ki
                        nc.tensor.matmul(
                            at_ps[:, trim:CHUNK],
                            v_t[:, ki * F_LOC + h * HD:
                                ki * F_LOC + (h + 1) * HD],
                            e_t[:, ki * CHUNK + trim:(ki + 1) * CHUNK],
                            start=(term == 0 and ki == 0),
                            stop=(term == 1 and ki == nki - 1),
                        )
            else:
                nki = 4 * c + 4
                npair = nki // 2
                v8k = v8_t[:].rearrange("p (k f) -> p k f", k=TT)
                vrk = vr_t[:].rearrange("p (k f) -> p k f", k=TT)
                ek = kview(e_t)
                hs = slice(h * HD, (h + 1) * HD)
                for kp in range(npair):
                    ptrim = 256 if kp == npair - 1 else 0
                    nc.tensor.matmul(
                        rs_ps[:, ptrim:CHUNK],
                        ones8[:].rearrange("p (k f) -> p k f", k=2),
                        ek[:, 2 * kp:2 * kp + 2, ptrim:CHUNK],
                        start=(kp == 0), stop=(kp == npair - 1),
                        perf_mode=DRM,
                    )
                nc.vector.reciprocal(ri_t[:], rs_ps[:])
                n = 2 * npair
                i = 0
                for vk in (v8k, vrk):
                    for kp in range(npair):
                        ptrim = 256 if kp == npair - 1 else 0
                        nc.tensor.matmul(
                            at_ps[:, ptrim:CHUNK],
                            vk[:, 2 * kp:2 * kp + 2, hs],
                            ek[:, 2 * kp:2 * kp + 2, ptrim:CHUNK],
                            start=(i == 0), stop=(i == n - 1),
                            perf_mode=DRM,
                        )
                        i += 1
            # normalize (DVE: PSUM) + fp8 value/residual split (Pool for
            # mid-kernel chunks — SBUF-only ops — DVE for the tail chunk)
            a8_t, ar_t = acts[4], acts[5]
            a16 = ripool.tile([128, CHUNK], BF16, tag="a16", bufs=3,
                              name="a16")
            sl = slice(h * S + c * CHUNK, h * S + c * CHUNK + CHUNK)
            nc.vector.tensor_tensor(a16[:], at_ps[:], ri_t[:], MULT)
            nc.gpsimd.tensor_copy(a8_t[:, sl], a16[:])
            nc.gpsimd.tensor_tensor(ar_t[:, sl], a16[:], a8_t[:, sl], SUB)

        def ot_group(b, c, acts, wo_t, t4, engines=("pool", "dve"),
                     ps=None, split_y=False, quarter_y=False):
            """fp8 DR output projection + y writeback for one token tile.

            3-term: y = (a8+ar)@wo8 + a8@wor, heads stacked on the DR
            pair dim.  1/32 rides in the y drain (DVE/Pool alternating)."""
            a8_t, ar_t = acts[4], acts[5]
            a8v = a8_t[:].rearrange("p (h s) -> p h s", h=H_LOC)
            arv = ar_t[:].rearrange("p (h s) -> p h s", h=H_LOC)
            wov = wo_t[:].rearrange("p (h t f) -> p h t f", h=H_LOC, t=2)
            tt = c * (CHUNK // 128) + t4
            ts4 = slice(tt * 128, (tt + 1) * 128)
            y_t = ypool.tile([128, D], BF16, tag="y", name="y_t")
            row0 = b * S + tt * 128
            for oc in range(D // CHUNK):
                ocs = slice(oc * CHUNK, (oc + 1) * CHUNK)
                if ps is None:
                    o_ps = pso.tile([128, CHUNK], F32, tag="pso",
                                    name="o_ps")
                else:
                    # tail: alternate the freed qkv/pv and outproj rings
                    # for an effective PSUM depth of 5
                    pp, tg = ((ps, qtag) if oc % 2 == 0
                              else (pso, "pso"))
                    o_ps = pp.tile([128, CHUNK], F32, tag=tg, name="o_ps")
                nc.tensor.matmul(
                    o_ps[:], a8v[:, :, ts4], wov[:, :, 0, ocs],
                    start=True, stop=False, perf_mode=DRM,
                )
                nc.tensor.matmul(
                    o_ps[:], arv[:, :, ts4], wov[:, :, 0, ocs],
                    start=False, stop=False, perf_mode=DRM,
                )
                nc.tensor.matmul(
                    o_ps[:], a8v[:, :, ts4], wov[:, :, 1, ocs],
                    start=False, stop=True, perf_mode=DRM,
                )
                eng = engines[oc % len(engines)]
                if eng == "dual":
                    # latency-critical tail: half on ACT, half on DVE in
                    # parallel (GPSIMD cannot read PSUM on TRN2)
                    h0 = slice(oc * CHUNK, oc * CHUNK + CHUNK // 2)
                    h1 = slice(oc * CHUNK + CHUNK // 2, (oc + 1) * CHUNK)
                    nc.scalar.activation(
                        y_t[:, h0], o_ps[:, 0:CHUNK // 2],
                        mybir.ActivationFunctionType.Copy,
                        bias=0.0, scale=1.0 / W_SCALE)
                    nc.vector.tensor_scalar(
                        y_t[:, h1], o_ps[:, CHUNK // 2:CHUNK],
                        1.0 / W_SCALE, None, op0=MULT)
                elif eng == "dve":
                    nc.vector.tensor_scalar(
                        y_t[:, ocs], o_ps[:], 1.0 / W_SCALE, None, op0=MULT)
                else:
                    nc.scalar.activation(
                        y_t[:, ocs], o_ps[:],
                        mybir.ActivationFunctionType.Copy,
                        bias=0.0, scale=1.0 / W_SCALE)
                if quarter_y:
                    nc.sync.dma_start(
                        y_d.ap()[row0:row0 + 128, ocs], y_t[:, ocs])
                elif split_y and oc == 1:
                    nc.sync.dma_start(
                        y_d.ap()[row0:row0 + 128, 0:D // 2],
                        y_t[:, 0:D // 2])
            # one full-row y write per token tile: 4x fewer DMA issues
            if split_y:
                nc.sync.dma_start(
                    y_d.ap()[row0:row0 + 128, D // 2:D],
                    y_t[:, D // 2:D])
            elif not quarter_y:
                nc.sync.dma_start(y_d.ap()[row0:row0 + 128, :], y_t[:])

        def new_acts():
            qT_t = actpool.tile([128, H_LOC * S], BF16, tag="actq",
                                bufs=2, name="qT_t")
            kT_t = actpool.tile([128, H_LOC * S], BF16, tag="actk",
                                bufs=2, name="kT_t")
            v8_t = actpool.tile([128, TT * F_LOC], FP8E4, tag="actv",
                                bufs=2, name="v8_t")
            vr_t = actpool.tile([128, TT * F_LOC], FP8E4, tag="actvr",
                                bufs=2, name="vr_t")
            a8_t = actpool.tile([128, H_LOC * S], FP8E4, tag="acta8",
                                bufs=2, name="a8_t")
            ar_t = actpool.tile([128, H_LOC * S], FP8E4, tag="actar",
                                bufs=2, name="ar_t")
            return (qT_t, kT_t, v8_t, vr_t, a8_t, ar_t)

        def load_wo():
            # woT [F_LOC, 2*D] (wo8 | wor) -> [128, H_LOC*2*D]; deferred
            # load so the DMA queue prioritizes x chunks during warmup
            wo_t = wpool.tile([128, H_LOC * 2 * D], FP8E4, tag="wo",
                              name="wo_t")
            nc.sync.dma_start(
                wo_t[:].rearrange("p (h f) -> p h f", h=H_LOC),
                woT_d.ap().rearrange("(h p) f -> p h f", p=128),
            )
            return wo_t

        def emit_body(first_iter=True):
            """Greedy list-scheduler: fine-grained units interleaved so
            the in-order PE stream never outruns ACT exp (scores), DVE
            normalize (outproj inputs), or the PSUM drain engines."""
            US = 1000.0
            # qkv group ids: 0,1=q(h0,h1) 2,3=k 4..7=v(t4) — q first so the
            # startup never waits on the later wk/wv transfers
            GQ = (("q", 0), ("q", 1), ("k", 0), ("k", 1),
                  ("v", 0), ("v", 1), ("v", 2), ("v", 3))
            QKV_PE = {"q": 2.56 * US, "k": 2.56 * US, "v": 1.28 * US}
            units = {}   # uid -> dict

            def add(uid, deps, kind, **kw):
                units[uid] = dict(kind=kind, deps=deps, done=False, **kw)

            chunks = [(b, c) for b in range(B) for c in range(NCH)]
            for b in range(B):
                for c in range(NCH):
                    prev = ([("qkv", b, c - 1, g) for g in range(8)]
                            if c > 0 else
                            ([("qkv", b - 1, NCH - 1, g)
                              for g in range(4, 8)] if b > 0 else []))
                    for g, (kd, idx) in enumerate(GQ):
                        add(("qkv", b, c, g), list(prev), "qkv",
                            b=b, c=c, g=kd, idx=idx,
                            pe=QKV_PE[kd], act=0.0)
                    nki = 4 * c + 4
                    ngrp = (nki + 2) // 3
                    for h in range(H_LOC):
                        for i in range(ngrp):
                            blk0 = 3 * i
                            nblk = min(3, nki - blk0)
                            kc = (blk0 + nblk - 1) // 4
                            deps = [("qkv", b, c, h)]        # q of head h
                            deps += [("qkv", b, cc, 2 + h)
                                     for cc in range(kc + 1)]
                            if i > 0:
                                deps.append(("sc", b, c, h, i - 1))
                            add(("sc", b, c, h, i), deps, "sc",
                                b=b, c=c, h=h, i=i, blk0=blk0, nblk=nblk,
                                kc=kc,
                                pe=nblk * 0.23 * US,
                                act=nblk * 0.55 * US)
                        deps = [("sc", b, c, h, i) for i in range(ngrp)]
                        deps += [("qkv", b, cc, 4 + t)
                                 for cc in range(c + 1) for t in range(4)]
                        add(("pv", b, c, h), deps, "pv", b=b, c=c, h=h,
                            pe=(1.6 if c == 0 else 0.5 + 0.47 * c) * US,
                            act=0.0)
                    for t4 in range(4):
                        deps = [("pv", b, c, h) for h in range(H_LOC)]
                        add(("ot", b, c, t4), deps, "ot",
                            b=b, c=c, t4=t4, pe=1.28 * US, act=0.0)
            # hold back the tail: the last chunk's outproj is gated by a
            # ~2.6us DVE normalize chain; keep two earlier outproj units
            # in reserve so the PE has fill during that window
            units[("ot", B - 1, NCH - 4, 3)]["deps"].append(
                ("sc", B - 1, NCH - 2, 1, 0))
            units[("ot", B - 1, NCH - 3, 2)]["deps"].append(
                ("sc", B - 1, NCH - 1, 0, 0))
            units[("ot", B - 1, NCH - 3, 3)]["deps"].append(
                ("sc", B - 1, NCH - 1, 1, 0))
            units[("ot", B - 1, NCH - 2, 2)]["deps"].append(
                ("pv", B - 1, NCH - 1, 0))
            units[("ot", B - 1, NCH - 2, 3)]["deps"].append(
                ("pv", B - 1, NCH - 1, 1))

            # ---- greedy emission with virtual engine clocks ----
            loaded = {}
            if first_iter:
                loaded[(0, 0)] = x_first
                loaded[(0, 1)] = x_second
            nloaded = [2 if first_iter else 0]
            acts_by_b = {}
            e_by = {}
            live_e8 = [0]
            wo_t = [None]
            pe_t = [0.0]
            act_t = [0.0]
            dve_t = [0.0]
            fin = {}     # uid -> virtual drain-finish time (DVE work)
            sc_fin = {}  # (b,c,h) -> virtual act finish of its exps

            def ensure_x(bc):
                if bc not in loaded:
                    loaded[bc] = load_x(*bc)
                    nloaded[0] = max(nloaded[0], chunks.index(bc) + 1)

            def stall_of(uid):
                """Virtual ns PE would wait on producers before this unit."""
                u = units[uid]
                need = 0.0
                if u["kind"] == "sc":
                    b, c, h = u["b"], u["c"], u["h"]
                    need = max(fin.get(("qkv", b, c, h), 0.0),
                               fin.get(("qkv", b, u["kc"], 2 + h), 0.0))
                elif u["kind"] == "pv":
                    need = sc_fin.get((u["b"], u["c"], u["h"]), 0.0)
                elif u["kind"] == "ot":
                    need = max(fin.get(("pv", u["b"], u["c"], h), 0.0)
                               for h in range(H_LOC))
                return max(0.0, need + 1100.0 - pe_t[0])

            DVE_NS = {"q": 660.0, "k": 660.0, "v": 780.0}

            def emit(uid):
                u = units[uid]
                pe_t[0] += stall_of(uid)
                if u["kind"] == "qkv":
                    b, c = u["b"], u["c"]
                    if b not in acts_by_b:
                        acts_by_b[b] = new_acts()
                    ensure_x((b, c))
                    if nloaded[0] < len(chunks):
                        ensure_x(chunks[nloaded[0]])
                    fuse = (b == 0 and c == 0 and u["g"] in ("q", "k")
                            and u["idx"] == 0)
                    if b == 0 and c == 0 and u["g"] in ("q", "k") \
                            and u["idx"] == 1:
                        pass  # emitted by the fused head-0 unit
                    else:
                        qkv_group(loaded[(b, c)], c, acts_by_b[b],
                                  u["g"], u["idx"], fused=fuse)
                    pe_t[0] += u["pe"]
                    dve_t[0] = max(dve_t[0], pe_t[0]) + DVE_NS[u["g"]]
                    fin[uid] = dve_t[0]
                elif u["kind"] == "sc":
                    b, c, h = u["b"], u["c"], u["h"]
                    if wo_t[0] is None:
                        wo_t[0] = load_wo()
                    if (b, c) not in e_by:
                        e_by[(b, c)] = new_e(c)
                        if c >= 1:
                            live_e8[0] += 2
                    sc_group(c, acts_by_b[b], h, u["blk0"],
                             e_by[(b, c)][h], u["nblk"])
                    pe_t[0] += u["pe"]
                    act_t[0] = max(act_t[0], pe_t[0]) + u["act"]
                    sc_fin[(b, c, h)] = act_t[0]
                elif u["kind"] == "pv":
                    b, c, h = u["b"], u["c"], u["h"]
                    pv_head(c, acts_by_b[b], e_by[(b, c)][h], h,
                            tail_pv=(b == B - 1 and c == NCH - 1))
                    if h == H_LOC - 1:
                        del e_by[(b, c)]
                        if c >= 1:
                            live_e8[0] -= 2
                    pe_t[0] += u["pe"]
                    dve_t[0] = max(dve_t[0], pe_t[0]) + 1500.0
                    fin[uid] = dve_t[0]
                else:
                    b, c = u["b"], u["c"]
                    last = (b == B - 1 and c >= NCH - 2)
                    # after the final pv, the qkv/pv PSUM ring is free —
                    # use it to deepen the tail outproj pipeline
                    tail2 = (b == B - 1
                             and (c == NCH - 1
                                  or (c == NCH - 2 and u["t4"] >= 2)))
                    engines = ("dve", "act")
                    if b == B - 1 and not last:
                        # keep ACT clear for the final chunks' exps
                        engines = ("dve", "dve")
                    elif tail2:
                        engines = ("act", "dve")
                    ot_group(b, c, acts_by_b[b], wo_t[0], u["t4"],
                             engines=engines,
                             ps=psq if tail2 else None,
                             split_y=(b == B - 1 and c >= NCH - 2))
                    pe_t[0] += u["pe"]
                    if not last:
                        dve_t[0] = max(dve_t[0], pe_t[0]) + 660.0
                u["done"] = True

            def ready(uid):
                u = units[uid]
                if u["done"]:
                    return False
                if any(not units[d]["done"] for d in u["deps"]):
                    return False
                if (u["kind"] == "sc" and u["i"] == 0 and u["c"] >= 1
                        and (u["b"], u["c"]) not in e_by
                        and live_e8[0] >= 4):
                    return False  # e8 pool would force a WAR stall
                return True

            nqkv_done = [0]
            while True:
                cands = [uid for uid in units if ready(uid)]
                if not cands:
                    break
                sc_c = [u for u in cands if units[u]["kind"] == "sc"]
                pv_c = [u for u in cands if units[u]["kind"] == "pv"]
                ot_c = [u for u in cands if units[u]["kind"] == "ot"]
                qk_c = [u for u in cands if units[u]["kind"] == "qkv"]
                pick = None
                # keep ACT fed when it is about to starve
                if sc_c and act_t[0] - pe_t[0] < 9.0 * US:
                    ns = [u for u in sc_c if stall_of(u) <= 0.0]
                    if ns:
                        pick = min(ns)
                if pick is None and pv_c:
                    ns = [u for u in pv_c if stall_of(u) <= 0.0]
                    if ns:
                        pick = min(ns)
                if pick is None and ot_c:
                    ns = [u for u in ot_c if stall_of(u) <= 0.0]
                    if ns:
                        pick = min(ns)
                if pick is None and qk_c:
                    pick = min(qk_c)
                if pick is None and cands:
                    # nothing stall-free: pick the least-stalling unit
                    pick = min(cands, key=lambda u: (stall_of(u), u))
                emit(pick)
                u = units[pick]
                if u["kind"] == "qkv":
                    nqkv_done[0] += 1
                    # x chunk fully consumed after its 8 groups
                    b, c = u["b"], u["c"]
                    if all(units[("qkv", b, c, g)]["done"]
                           for g in range(8)):
                        loaded.pop((b, c), None)

        if reps is None:
            emit_body()
        else:
            with tc.For_i(0, reps, 1):
                emit_body(first_iter=False)

        pools = [pso, pss, psq, ripool, ypool, actpool, xkpool,
                 wpool, cpool]
        seen = set()
        for p in pools:
            if id(p) not in seen:
                seen.add(id(p))
                p.release()

    nc.compile()
    return nc


def _get_nc(reps=None):
    key = ("nc", reps)
    if key not in _CACHE:
        _CACHE[key] = _build(reps)
    return _CACHE[key]


def _fp8(a):
    import ml_dtypes
    return np.ascontiguousarray(a).astype(ml_dtypes.float8_e4m3)


def _bf16(a):
    import ml_dtypes
    return np.ascontiguousarray(a).astype(ml_dtypes.bfloat16)


def make_in_maps(x, wq, bq, wk, bk, wv, bv, wo):
    x = np.asarray(x, dtype=np.float32)
    xT = np.ascontiguousarray(x.reshape(B * S, D).T)  # [D, B*S]
    x8T = _fp8(xT)
    xrT = _fp8(xT - x8T.astype(np.float32))
    # rows (k-tile, partition); cols (chunk, two, f): every DRAM row
    # holds (x8 | xr) of one chunk contiguously -> 1KiB DMA descriptors
    nch = B * NCH
    xiT = np.empty((D, nch, 2, CHUNK), dtype=x8T.dtype)
    xiT[:, :, 0] = x8T.reshape(D, nch, CHUNK)
    xiT[:, :, 1] = xrT.reshape(D, nch, CHUNK)
    xiT = np.ascontiguousarray(xiT.reshape(D, 2 * B * S))

    in_maps = []
    for i in range(N_CORES):
        fs = slice(i * F_LOC, (i + 1) * F_LOC)
        # wo pair: [F_LOC, 2*D] = (wo8 | wor) of 32*wo^T interleaved
        woT = W_SCALE * np.asarray(wo, np.float32)[:, fs].T  # [F_LOC, D]
        wo8 = _fp8(woT)
        wor = _fp8(woT - wo8.astype(np.float32))
        wopack = np.stack([wo8, wor], axis=1).reshape(F_LOC, 2 * D)
        m = {
            "xiT": xiT,
            "woT": np.ascontiguousarray(wopack),
            "bq2": np.ascontiguousarray(
                np.asarray(bq)[fs].reshape(H_LOC, HD).T.astype(np.float32)),
            "ones": _fp8(np.full((128, 256), W_SCALE, dtype=np.float32)),
        }
        for nm, w in (("wq", wq), ("wk", wk), ("wv", wv)):
            wT = W_SCALE * np.asarray(w, dtype=np.float32)[fs, :].T
            w8 = _fp8(wT)
            wr = _fp8(wT - w8.astype(np.float32))
            # rows (k-tile, partition); cols (two, f) -> 512B descriptors
            m[nm + "pT"] = np.ascontiguousarray(
                np.stack([w8, wr], axis=1).reshape(D, 2 * F_LOC))
        in_maps.append(m)
    return in_maps


def kernel(x, wq, bq, wk, bk, wv, bv, wo, bo):
    from concourse.bass_utils import run_bass_kernel_spmd

    nc = _get_nc()
    in_maps = make_in_maps(x, wq, bq, wk, bk, wv, bv, wo)
    res = run_bass_kernel_spmd(nc, in_maps, core_ids=list(range(N_CORES)),
                               trace=False)
    y = np.zeros((B * S, D), dtype=np.float32)
    for i in range(N_CORES):
        y += res.results[i]["y"].astype(np.float32)
    # v bias folded through the output projection, plus bo
    y += (np.asarray(wo, dtype=np.float32) @ np.asarray(bv, np.float32)
          + np.asarray(bo, np.float32))[None, :]
    return y.reshape(B, S, D)
